# revision 32
# baseline (speedup 1.0000x reference)
"""Multi-head self-attention with RoPE on 8 Trainium2 NeuronCores.

Problem: B=2, S=2048, D_MODEL=2048, 16 heads x d_k=128, causal, RoPE on Q/K.

Sharding (hardcoded): core c -> batch b=c//4, head group g=c%4 (heads 4g..4g+3).
Data parallel on batch, tensor parallel on heads; q/k/v projections column-
sharded, output projection row-sharded with the partial sums reduced on host.

Device kernel, three passes (bf16/fp16 operands, f32 PSUM accum), tuned so the
PE streams nothing but real matmul columns:
  pass1: V proj (all 4 heads) + Q/K proj pair0 fused over one sweep of x.
  pass2: pair0 attention with Q/K proj pair1 matmuls interleaved 1:1 into the
         attention k-tile slots (the proj matmuls fill the PE while the scalar
         engine's exp keeps pace).
  pass3: pair1 attention with the output projection interleaved the same way.
  Softmax denominator: exp tiles are accumulated on the vector engine (fp16);
  one all-ones [128,128] stationary matmul then reduces across partitions AND
  broadcasts the sums to every partition in a single 512-col pass (213ns on
  the PE vs 3.5us for a gpsimd partition reduce).  Each head's denominator/
  normalize tail is deferred into the next head's slot stream so the PSUM
  bank handoff never stalls the PE.
  Causal mask: 0/1 upper-triangular multiply on the diagonal 128x128 block of
  the exp'd tile (vector engine), replacing the -1e9 mask matmul.
  All PSUM->SBUF copies (V, output staging) run on the vector engine so the
  scalar engine does nothing but exp.
  A short zero matmul warm-up stream un-throttles the PE HAM clock gate while
  the initial weight DMAs land.
"""

import sys

sys.path.insert(0, "/opt/trn_rl_repo")

import math

import ml_dtypes
import numpy as np

import concourse.bass as bass
import concourse.mybir as mybir
from concourse import bass_isa
import concourse.tile as tile
from concourse import bacc
from concourse.bass_utils import run_bass_kernel_spmd

f32 = mybir.dt.float32
bf16 = mybir.dt.bfloat16
fp16 = mybir.dt.float16

B = 2
S = 2048
D = 2048
H = 16
DK = 128
H_CORE = 4  # heads per core
DL = H_CORE * DK  # local feature dim 512
ET = D // 128  # 16 e-tiles (contraction over d_model)
QC = S // 512  # 4 q-chunks
THETA = 10000.0
SCALE = 1.0 / math.sqrt(DK)

N_CORES = 8


def _build():
    nc = bacc.Bacc("TRN2", target_bir_lowering=False, debug=False)

    xT_d = nc.dram_tensor("xT", [D, S], bf16, kind="ExternalInput")
    # per-pair flattened qk weights: cols [p*4096 + et*256 + c], c 0:128 even
    # dims (qe stationary), 128:256 odd dims (qo)
    wqf_d = nc.dram_tensor("wqf", [128, 2 * ET * 256], bf16, kind="ExternalInput")
    wkf_d = nc.dram_tensor("wkf", [128, 2 * ET * 256], bf16, kind="ExternalInput")
    # flattened v weights: cols [et*512 + dl]
    wvf_d = nc.dram_tensor("wvf", [128, ET * DL], bf16, kind="ExternalInput")
    woT_d = nc.dram_tensor("woT", [DL, D], bf16, kind="ExternalInput")
    cosT_d = nc.dram_tensor("cosT", [64, S], f32, kind="ExternalInput")
    sinT_d = nc.dram_tensor("sinT", [64, S], f32, kind="ExternalInput")
    tri_d = nc.dram_tensor("tri01", [128, 128], fp16, kind="ExternalInput")
    outT_d = nc.dram_tensor("outT", [D, S], bf16, kind="ExternalOutput")

    Exp = mybir.ActivationFunctionType.Exp

    with tile.TileContext(nc) as tc:
      with tc.tile_pool(name="const", bufs=1) as const, \
           tc.tile_pool(name="persist", bufs=1) as persist, \
           tc.tile_pool(name="qkp", bufs=1) as qkp, \
           tc.tile_pool(name="wqp", bufs=2) as wqp, \
           tc.tile_pool(name="wkp", bufs=2) as wkp, \
           tc.tile_pool(name="xsp", bufs=20) as xsp, \
           tc.tile_pool(name="ropet", bufs=1) as ropet, \
           tc.tile_pool(name="ptp", bufs=6) as ptp, \
           tc.tile_pool(name="accp", bufs=2) as accp, \
           tc.tile_pool(name="denp", bufs=2) as denp, \
           tc.tile_pool(name="stgp", bufs=8) as stgp, \
           tc.tile_pool(name="wop", bufs=1) as wop, \
           tc.tile_pool(name="psum", bufs=1, space="PSUM") as psum:

        V = [persist.tile([128, DL], fp16, tag=f"v{st}", name=f"v{st}")
             for st in range(ET)]
        OT = [persist.tile([DK, S], bf16, tag=f"ot{h}", name=f"ot{h}")
              for h in range(H_CORE)]

        cos2 = const.tile([128, S], f32, tag="cos2", name="cos2")
        sin2 = const.tile([128, S], f32, tag="sin2", name="sin2")
        tri01 = const.tile([128, 128], fp16, tag="tri01", name="tri01")
        zstat = const.tile([128, 128], bf16, tag="zstat", name="zstat")
        zmov = const.tile([128, 512], bf16, tag="zmov", name="zmov")
        ones_f = const.tile([128, 128], f32, tag="ones_f", name="ones_f")
        ones16 = const.tile([128, 128], fp16, tag="ones16", name="ones16")

        def load_wqk(p, chunks=4, eng=None):
            wq_all = wqp.tile([128, ET * 256], bf16, tag="wq", name="wq")
            wk_all = wkp.tile([128, ET * 256], bf16, tag="wk", name="wk")
            cw = 4096 // chunks
            eng = eng or nc.gpsimd
            for kk in range(chunks):
                cs = slice(kk * cw, (kk + 1) * cw)
                ds = slice(p * 4096 + kk * cw, p * 4096 + (kk + 1) * cw)
                eng.dma_start(wq_all[:, cs], wqf_d[:, ds])
                eng.dma_start(wk_all[:, cs], wkf_d[:, ds])
            return wq_all, wk_all

        def load_x(qc):
            """Per-chunk x tiles, alternating sync/gpsimd queues by e-tile
            parity so arrivals match the et consumption order."""
            qs = slice(qc * 512, (qc + 1) * 512)
            xts = []
            for et in range(ET):
                xt = xsp.tile([128, 512], bf16, tag="xs", name="xs")
                eng = nc.sync if et % 2 == 0 else nc.gpsimd
                eng.dma_start(xt[:], xT_d[et * 128:(et + 1) * 128, qs])
                xts.append(xt)
            return xts

        def rope(dst0, dst1, ev, od, qs):
            """ev/od: PSUM accumulators (128,512), rows [hA;hB].

            Reads ev fully before od so the PSUM banks free in order for the
            next q-chunk's projection.
            """
            c = cos2[:, qs]
            sn = sin2[:, qs]
            m1 = ropet.tile([128, 512], bf16, tag="m1", name="m1")
            n1 = ropet.tile([128, 512], bf16, tag="n1", name="n1")
            nc.vector.tensor_mul(m1[:], ev[:], c)
            nc.vector.tensor_mul(n1[:], ev[:], sn)
            m2 = ropet.tile([128, 512], bf16, tag="m2", name="m2")
            n2 = ropet.tile([128, 512], bf16, tag="n2", name="n2")
            nc.vector.tensor_mul(m2[:], od[:], sn)
            nc.vector.tensor_mul(n2[:], od[:], c)
            nc.vector.tensor_sub(dst0[0:64, qs], m1[0:64, :], m2[0:64, :])
            nc.vector.tensor_sub(dst1[0:64, qs], m1[64:128, :], m2[64:128, :])
            nc.vector.tensor_add(dst0[64:128, qs], n1[0:64, :], n2[0:64, :])
            nc.vector.tensor_add(dst1[64:128, qs], n1[64:128, :], n2[64:128, :])

        def proj_chunk1(qc, wq_all, wk_all, wv_all, xts):
            """Pass1: QK pair0 + V projection matmuls for one q-chunk.

            xts: this chunk's 16 x tiles [128,512].

            Chunk 0 runs half-major (e-tiles 0-7 fully used before 8-15, so
            the matmuls track the DMA arrival order); later chunks run
            accumulator-major with the rope combines emitted as soon as each
            even/odd accumulator pair closes, spreading the DVE work across
            the chunk instead of bunching it at the boundary.
            """
            qs = slice(qc * 512, (qc + 1) * 512)
            qe = psum.tile([128, 512], f32, tag="t0", name="qe")
            qo = psum.tile([128, 512], f32, tag="t1", name="qo")
            ke = psum.tile([128, 512], f32, tag="t2", name="ke")
            ko = psum.tile([128, 512], f32, tag="t3", name="ko")
            vacc = [psum.tile([128, DL], f32, tag=f"t{4 + i}",
                              name=f"vacc{i}") for i in range(4)]
            accs = ((qe, wq_all, 0), (qo, wq_all, 128),
                    (ke, wk_all, 0), (ko, wk_all, 128))
            if qc == 0:
                for half in range(2):
                    ets = range(half * 8, half * 8 + 8)
                    for acc, w_all, coff in accs:
                        for et in ets:
                            nc.tensor.matmul(
                                acc[:],
                                w_all[:, et * 256 + coff:
                                      et * 256 + coff + 128],
                                xts[et][:],
                                start=(et == 0), stop=(et == ET - 1),
                            )
                rope(QT0[0], QT0[1], qe, qo, qs)
                rope(KT0[0], KT0[1], ke, ko, qs)
            else:
                for ai, (acc, w_all, coff) in enumerate(accs):
                    for et in range(ET):
                        nc.tensor.matmul(
                            acc[:],
                            w_all[:, et * 256 + coff: et * 256 + coff + 128],
                            xts[et][:],
                            start=(et == 0), stop=(et == ET - 1),
                        )
                    if ai == 1:
                        rope(QT0[0], QT0[1], qe, qo, qs)
                    elif ai == 3:
                        rope(KT0[0], KT0[1], ke, ko, qs)
            for sl in range(4):
                for et in range(ET):
                    nc.tensor.matmul(
                        vacc[sl][:],
                        xts[et][:, sl * 128:(sl + 1) * 128],
                        wv_all[:, et * DL:(et + 1) * DL],
                        start=(et == 0), stop=(et == ET - 1),
                    )
            return vacc

        def proj_qk_mms(qc, wq_all, wk_all, QTp, KTp, xts):
            """Pass2 filler: QK pair1 projection, yielding after each matmul.

            Two PSUM banks only (t0/t1), with the rope combines emitted inline
            once each even/odd accumulator pair closes; t2/t3 stay free for
            the attention denominator matmuls.
            """
            qs = slice(qc * 512, (qc + 1) * 512)
            for w_all, dsts in ((wq_all, QTp), (wk_all, KTp)):
                ev = psum.tile([128, 512], f32, tag="t0", name="ev")
                od = psum.tile([128, 512], f32, tag="t1", name="od")
                for acc, coff in ((ev, 0), (od, 128)):
                    for et in range(ET):
                        nc.tensor.matmul(
                            acc[:],
                            w_all[:, et * 256 + coff: et * 256 + coff + 128],
                            xts[et][:],
                            start=(et == 0), stop=(et == ET - 1),
                            skip_group_check=True,
                        )
                        yield
                rope(dsts[0], dsts[1], ev, od, qs)

        def outproj_mms(qcp, wo_h):
            """Pass3 filler: output projection for chunk qcp, yield per matmul."""
            qs = slice(qcp * 512, (qcp + 1) * 512)
            for et in range(ET):
                facc = psum.tile([128, 512], f32, tag=f"t{et % 2}",
                                 name="facc")
                for hh in range(H_CORE):
                    nc.tensor.matmul(
                        facc[:],
                        wo_h[hh][:, et * 128:(et + 1) * 128],
                        OT[hh][:, qs],
                        start=(hh == 0), stop=(hh == H_CORE - 1),
                        skip_group_check=True,
                    )
                    yield
                st = stgp.tile([128, 512], bf16, tag="stg", name="st")
                nc.vector.tensor_copy(st[:], facc[:])
                eng = nc.sync if et % 2 == 0 else nc.gpsimd
                eng.dma_start(outT_d[et * 128:(et + 1) * 128, qs], st[:])

        def attn_gen(QTp, KTp, p, hi, qc, oacc_tag, den_tag, tails):
            """Attention for head (pair p, index hi) on q-chunk qc.

            Yields once per k-tile so the caller can interleave one filler
            matmul per slot.  Softmax denominator: DVE fp16 accumulation of
            the exp tiles, then one all-ones stationary matmul that reduces
            across partitions and broadcasts in the same pass.  The final
            den/recip/normalize tail is appended to `tails` for the caller
            to emit a few slots into the NEXT head (PSUM handoff slack).
            """
            h = 2 * p + hi
            nkt = 4 * qc + 4
            LAG = 2
            qs = slice(qc * 512, (qc + 1) * 512)
            oacc = psum.tile([128, 512], f32, tag=oacc_tag, name="oacc")
            ptacc = accp.tile([128, 512], fp16, tag="ptacc", name="ptacc")
            pts = {}

            def consume(kt):
                j = kt - 4 * qc
                o = 128 * j if j > 0 else 0
                cs = slice(o, 512)
                pt = pts.pop(kt)
                nc.tensor.matmul(
                    oacc[:, cs], V[kt][:, h * 128:(h + 1) * 128], pt[:, cs],
                    start=(kt == 0), stop=(kt == nkt - 1),
                    skip_group_check=True,
                )

            for kt in range(nkt):
                j = kt - 4 * qc
                o = 128 * j if j > 0 else 0
                cs = slice(o, 512)
                sps = psum.tile(
                    [128, 512], f32, tag=("t4" if kt % 2 == 0 else "t5"),
                    name="sps",
                )
                nc.tensor.matmul(
                    sps[:, cs],
                    KTp[hi][:, kt * 128:(kt + 1) * 128],
                    QTp[hi][:, qc * 512 + o:(qc + 1) * 512],
                    start=True, stop=True,
                    skip_group_check=True,
                )
                pt = ptp.tile([128, 512], fp16, tag="pt", name="pt")
                nc.scalar.activation(pt[:, cs], sps[:, cs], Exp, scale=SCALE)
                if j >= 0:
                    mb = slice(o, o + 128)
                    nc.vector.tensor_mul(pt[:, mb], pt[:, mb], tri01[:])
                if kt == 0:
                    nc.vector.tensor_copy(ptacc[:], pt[:])
                else:
                    nc.vector.tensor_add(ptacc[:, cs], ptacc[:, cs], pt[:, cs])
                pts[kt] = pt
                if kt >= LAG:
                    consume(kt - LAG)
                yield
            for kt in range(max(0, nkt - LAG), nkt):
                consume(kt)

            def tail():
                denb = psum.tile([128, 512], f32, tag=den_tag, name="denb")
                nc.tensor.matmul(denb[:], ones16[:], ptacc[:],
                                 start=True, stop=True, skip_group_check=True)
                rec = denp.tile([128, 512], f32, tag="rec", name="rec")
                nc.vector.reciprocal_approx_fast(rec[:], denb[:])
                nc.vector.tensor_mul(OT[h][:, qs], oacc[:], rec[:])

            tails.append(tail)

        # ---- warm-up: un-throttle the PE HAM gate during the initial DMAs --
        nc.vector.memset(zstat[:], 0.0)
        nc.vector.memset(zmov[:], 0.0)
        nc.vector.memset(ones_f[:], 1.0)
        nc.vector.tensor_copy(ones16[:], ones_f[:])
        for w in range(4):
            wps = psum.tile([128, 512], f32, tag="t7", name="warm")
            nc.tensor.matmul(wps[:], zstat[:], zmov[:], start=True, stop=True,
                             skip_group_check=True)

        # ---- pass1: V + QK pair0 over one sweep of x ---------------------
        QT0 = [qkp.tile([DK, S], bf16, tag=f"qt0{i}", name=f"qt0{i}")
               for i in range(2)]
        KT0 = [qkp.tile([DK, S], bf16, tag=f"kt0{i}", name=f"kt0{i}")
               for i in range(2)]
        def load_cossin(qc, eng):
            qs = slice(qc * 512, (qc + 1) * 512)
            eng.dma_start(cos2[0:64, qs], cosT_d[:, qs])
            eng.dma_start(cos2[64:128, qs], cosT_d[:, qs])
            eng.dma_start(sin2[0:64, qs], sinT_d[:, qs])
            eng.dma_start(sin2[64:128, qs], sinT_d[:, qs])

        with tc.tile_pool(name="wvp", bufs=1) as wvp:
            # scalar queue: wq0/wk0 interleaved with wv in consumption order
            # (first matmul needs wq0 chunk0; the V matmuls run ~8us later)
            wq0 = wqp.tile([128, ET * 256], bf16, tag="wq", name="wq")
            wk0 = wkp.tile([128, ET * 256], bf16, tag="wk", name="wk")
            wv_all = wvp.tile([128, ET * DL], bf16, tag="wv", name="wv")
            for kk in range(4):
                ws = slice(kk * 1024, (kk + 1) * 1024)
                nc.scalar.dma_start(wq0[:, ws], wqf_d[:, ws])
                nc.scalar.dma_start(wk0[:, ws], wkf_d[:, ws])
                vs = slice(kk * 2048, (kk + 1) * 2048)
                nc.scalar.dma_start(wv_all[:, vs], wvf_d[:, vs])
            xts0 = load_x(0)
            nc.scalar.dma_start(tri01[:], tri_d[:, :])

            wq1 = wk1 = None
            xts = xts0
            Copy = mybir.ActivationFunctionType.Copy
            for qc in range(QC):
                load_cossin(qc, nc.sync)
                vacc = proj_chunk1(qc, wq0, wk0, wv_all, xts)
                if qc + 1 < QC:
                    xts = load_x(qc + 1)
                for sl in range(4):
                    nc.scalar.activation(V[qc * 4 + sl][:], vacc[sl][:], Copy)
                if qc == 0:
                    wq1, wk1 = load_wqk(1)

        # ---- pass2: pair0 attention with QK pair1 proj interleaved -------
        QT1 = [qkp.tile([DK, S], bf16, tag=f"qt1{i}", name=f"qt1{i}")
               for i in range(2)]
        KT1 = [qkp.tile([DK, S], bf16, tag=f"kt1{i}", name=f"kt1{i}")
               for i in range(2)]
        wo_h = []
        for hh in range(H_CORE):
            wt = wop.tile([128, D], bf16, tag=f"wo{hh}", name=f"wo{hh}")
            nc.scalar.dma_start(wt[:, 0:1024], woT_d[hh * 128:(hh + 1) * 128, 0:1024])
            nc.scalar.dma_start(wt[:, 1024:2048], woT_d[hh * 128:(hh + 1) * 128, 1024:2048])
            wo_h.append(wt)

        tails = []

        def drive(gen, filler):
            n = 0
            for _ in gen:
                next(filler, None)
                n += 1
                if n == 3 and tails:
                    tails.pop(0)()

        def drain(filler):
            n = 0
            for _ in filler:
                n += 1
                if n == 3 and tails:
                    tails.pop(0)()
            while tails:
                tails.pop(0)()

        for qc in range(QC):
            xts = load_x(qc)
            filler = proj_qk_mms(qc, wq1, wk1, QT1, KT1, xts)
            for hi in range(2):
                drive(attn_gen(QT0, KT0, 0, hi, qc,
                               "t7" if hi == 0 else "t6",
                               "t2" if hi == 0 else "t3", tails), filler)
            drain(filler)

        # ---- pass3: pair1 attention with output projection interleaved ---
        for qc in range(QC):
            filler = outproj_mms(qc - 1, wo_h) if qc >= 1 else iter(())
            for hi in range(2):
                drive(attn_gen(QT1, KT1, 1, hi, qc,
                               "t7" if hi == 0 else "t6",
                               "t2" if hi == 0 else "t3", tails), filler)
            drain(filler)
        for _ in outproj_mms(QC - 1, wo_h):
            pass

    return nc


_NC = None


def _get_nc():
    global _NC
    if _NC is None:
        _NC = _build()
        _NC.compile()
    return _NC


def _rope_perm_rows():
    """Row permutation applied to wq/wk for one core's 4 heads.

    Per head-pair p: [hA even dims, hB even dims, hA odd dims, hB odd dims]
    so the device sees even/odd deinterleaved, pair-stacked projections.
    Returns indices into the local (4*DK,) head-row block.
    """
    idx = []
    for p in range(2):
        ha, hb = 2 * p, 2 * p + 1
        idx.extend(ha * DK + np.arange(0, DK, 2))
        idx.extend(hb * DK + np.arange(0, DK, 2))
        idx.extend(ha * DK + np.arange(1, DK, 2))
        idx.extend(hb * DK + np.arange(1, DK, 2))
    return np.asarray(idx)


def _host_tables(positions):
    """cos/sin tables (64, S) float32, matching the fp32 reference math."""
    dim_idx = np.arange(0, DK, 2, dtype=np.float32)
    freqs = np.float32(THETA) ** (dim_idx / np.float32(DK))
    angles = positions.astype(np.float32)[:, None] / freqs[None, :]  # (S, 64)
    return (
        np.ascontiguousarray(np.cos(angles).T.astype(np.float32)),
        np.ascontiguousarray(np.sin(angles).T.astype(np.float32)),
    )


def _flat_qk(wT):
    """(D, DL) -> (128, 2*ET*256): cols [p*4096 + et*256 + c]."""
    a = wT.reshape(ET, 128, DL)  # (et, p, dl)
    out = np.empty((128, 2 * ET * 256), dtype=wT.dtype)
    for p in range(2):
        blk = a[:, :, p * 256:(p + 1) * 256]  # (et, 128, 256)
        out[:, p * ET * 256:(p + 1) * ET * 256] = (
            blk.transpose(1, 0, 2).reshape(128, ET * 256)
        )
    return out


def _flat_v(wT):
    """(D, DL) -> (128, ET*DL): cols [et*512 + dl]."""
    a = wT.reshape(ET, 128, DL)
    return np.ascontiguousarray(a.transpose(1, 0, 2).reshape(128, ET * DL))


def _make_in_maps(inputs):
    x = np.asarray(inputs["x"], dtype=np.float32)
    wq = np.asarray(inputs["wq"], dtype=np.float32)
    wk = np.asarray(inputs["wk"], dtype=np.float32)
    wv = np.asarray(inputs["wv"], dtype=np.float32)
    wo = np.asarray(inputs["wo"], dtype=np.float32)
    token_positions = np.asarray(inputs["token_positions"])

    perm = _rope_perm_rows()
    bfc = ml_dtypes.bfloat16

    r = np.arange(128)
    tri01 = (r[:, None] <= r[None, :]).astype(np.float16)

    in_maps = []
    for c in range(N_CORES):
        b = c // 4
        g = c % 4
        rows = slice(g * DL, (g + 1) * DL)
        cosT, sinT = _host_tables(token_positions[b])
        in_maps.append(
            {
                "xT": np.ascontiguousarray(x[b].T).astype(bfc),
                "wqf": _flat_qk(wq[rows][perm].T).astype(bfc),
                "wkf": _flat_qk(wk[rows][perm].T).astype(bfc),
                "wvf": _flat_v(wv[rows].T).astype(bfc),
                "woT": np.ascontiguousarray(wo[:, rows].T).astype(bfc),
                "cosT": cosT,
                "sinT": sinT,
                "tri01": tri01,
            }
        )
    return in_maps


def kernel(x, wq, wk, wv, wo, token_positions):
    nc = _get_nc()
    in_maps = _make_in_maps(
        {
            "x": x,
            "wq": wq,
            "wk": wk,
            "wv": wv,
            "wo": wo,
            "token_positions": token_positions,
        }
    )
    res = run_bass_kernel_spmd(nc, in_maps, list(range(N_CORES)))

    out = np.zeros((B, S, D), dtype=np.float32)
    for c in range(N_CORES):
        out[c // 4] += res.results[c]["outT"].astype(np.float32).T
    return out


# revision 34
# speedup vs baseline: 1.0401x; 1.0401x over previous
"""Multi-head self-attention with RoPE on 8 Trainium2 NeuronCores.

Problem: B=2, S=2048, D_MODEL=2048, 16 heads x d_k=128, causal, RoPE on Q/K.

Sharding (hardcoded): core c -> batch b=c//4, head group g=c%4 (heads 4g..4g+3).
Data parallel on batch, tensor parallel on heads; q/k/v projections column-
sharded, output projection row-sharded with the partial sums reduced on host.

Device kernel, three passes (bf16/fp16 operands, f32 PSUM accum), tuned so the
PE streams nothing but real matmul columns:
  pass1: V proj (all 4 heads) + Q/K proj pair0 fused over one sweep of x.
  pass2: pair0 attention with Q/K proj pair1 matmuls interleaved 1:1 into the
         attention k-tile slots (the proj matmuls fill the PE while the scalar
         engine's exp keeps pace).
  pass3: pair1 attention with the output projection interleaved the same way.
  Softmax denominator: exp tiles are accumulated on the vector engine (fp16);
  one all-ones [128,128] stationary matmul then reduces across partitions AND
  broadcasts the sums to every partition in a single 512-col pass (213ns on
  the PE vs 3.5us for a gpsimd partition reduce).  Each head's denominator/
  normalize tail is deferred into the next head's slot stream so the PSUM
  bank handoff never stalls the PE.
  Causal mask: 0/1 upper-triangular multiply on the diagonal 128x128 block of
  the exp'd tile (vector engine), replacing the -1e9 mask matmul.
  All PSUM->SBUF copies (V, output staging) run on the vector engine so the
  scalar engine does nothing but exp.
  A short zero matmul warm-up stream un-throttles the PE HAM clock gate while
  the initial weight DMAs land.
"""

import sys

sys.path.insert(0, "/opt/trn_rl_repo")

import math

import ml_dtypes
import numpy as np

import concourse.bass as bass
import concourse.mybir as mybir
from concourse import bass_isa
import concourse.tile as tile
from concourse import bacc
from concourse.bass_utils import run_bass_kernel_spmd

f32 = mybir.dt.float32
bf16 = mybir.dt.bfloat16
fp16 = mybir.dt.float16

B = 2
S = 2048
D = 2048
H = 16
DK = 128
H_CORE = 4  # heads per core
DL = H_CORE * DK  # local feature dim 512
ET = D // 128  # 16 e-tiles (contraction over d_model)
QC = S // 512  # 4 q-chunks
THETA = 10000.0
SCALE = 1.0 / math.sqrt(DK)

N_CORES = 8


def _build():
    nc = bacc.Bacc("TRN2", target_bir_lowering=False, debug=False)

    xT_d = nc.dram_tensor("xT", [D, S], bf16, kind="ExternalInput")
    # per-pair flattened qk weights: cols [p*4096 + et*256 + c], c 0:128 even
    # dims (qe stationary), 128:256 odd dims (qo)
    wqf_d = nc.dram_tensor("wqf", [128, 2 * ET * 256], bf16, kind="ExternalInput")
    wkf_d = nc.dram_tensor("wkf", [128, 2 * ET * 256], bf16, kind="ExternalInput")
    # flattened v weights: cols [et*512 + dl]
    wvf_d = nc.dram_tensor("wvf", [128, ET * DL], bf16, kind="ExternalInput")
    woT_d = nc.dram_tensor("woT", [DL, D], bf16, kind="ExternalInput")
    cosT_d = nc.dram_tensor("cosT", [64, S], f32, kind="ExternalInput")
    sinT_d = nc.dram_tensor("sinT", [64, S], f32, kind="ExternalInput")
    tri_d = nc.dram_tensor("tri01", [128, 128], fp16, kind="ExternalInput")
    outT_d = nc.dram_tensor("outT", [D, S], bf16, kind="ExternalOutput")

    Exp = mybir.ActivationFunctionType.Exp

    with tile.TileContext(nc) as tc:
      with tc.tile_pool(name="const", bufs=1) as const, \
           tc.tile_pool(name="persist", bufs=1) as persist, \
           tc.tile_pool(name="qkp", bufs=1) as qkp, \
           tc.tile_pool(name="wqp", bufs=2) as wqp, \
           tc.tile_pool(name="wkp", bufs=2) as wkp, \
           tc.tile_pool(name="xsp", bufs=32) as xsp, \
           tc.tile_pool(name="ropet", bufs=1) as ropet, \
           tc.tile_pool(name="ptp", bufs=6) as ptp, \
           tc.tile_pool(name="accp", bufs=2) as accp, \
           tc.tile_pool(name="denp", bufs=2) as denp, \
           tc.tile_pool(name="stgp", bufs=8) as stgp, \
           tc.tile_pool(name="wop", bufs=1) as wop, \
           tc.tile_pool(name="psum", bufs=1, space="PSUM") as psum:

        V = [persist.tile([128, DL], fp16, tag=f"v{st}", name=f"v{st}")
             for st in range(ET)]
        OT = [persist.tile([DK, S], bf16, tag=f"ot{h}", name=f"ot{h}")
              for h in range(H_CORE)]

        cos2 = const.tile([128, S], f32, tag="cos2", name="cos2")
        sin2 = const.tile([128, S], f32, tag="sin2", name="sin2")
        tri01 = const.tile([128, 128], fp16, tag="tri01", name="tri01")
        zstat = const.tile([128, 128], bf16, tag="zstat", name="zstat")
        zmov = const.tile([128, 512], bf16, tag="zmov", name="zmov")
        ones_f = const.tile([128, 128], f32, tag="ones_f", name="ones_f")
        ones16 = const.tile([128, 128], fp16, tag="ones16", name="ones16")

        def load_wqk(p, chunks=4, eng=None):
            wq_all = wqp.tile([128, ET * 256], bf16, tag="wq", name="wq")
            wk_all = wkp.tile([128, ET * 256], bf16, tag="wk", name="wk")
            cw = 4096 // chunks
            eng = eng or nc.gpsimd
            for kk in range(chunks):
                cs = slice(kk * cw, (kk + 1) * cw)
                ds = slice(p * 4096 + kk * cw, p * 4096 + (kk + 1) * cw)
                eng.dma_start(wq_all[:, cs], wqf_d[:, ds])
                eng.dma_start(wk_all[:, cs], wkf_d[:, ds])
            return wq_all, wk_all

        def load_x(qc):
            """Per-chunk x tiles, alternating sync/gpsimd queues by e-tile
            parity so arrivals match the et consumption order."""
            qs = slice(qc * 512, (qc + 1) * 512)
            xts = []
            for et in range(ET):
                xt = xsp.tile([128, 512], bf16, tag="xs", name="xs")
                eng = nc.sync if et % 2 == 0 else nc.gpsimd
                eng.dma_start(xt[:], xT_d[et * 128:(et + 1) * 128, qs])
                xts.append(xt)
            return xts

        def rope(dst0, dst1, ev, od, qs):
            """ev/od: PSUM accumulators (128,512), rows [hA;hB].

            Reads ev fully before od so the PSUM banks free in order for the
            next q-chunk's projection.
            """
            c = cos2[:, qs]
            sn = sin2[:, qs]
            m1 = ropet.tile([128, 512], bf16, tag="m1", name="m1")
            n1 = ropet.tile([128, 512], bf16, tag="n1", name="n1")
            nc.vector.tensor_mul(m1[:], ev[:], c)
            nc.vector.tensor_mul(n1[:], ev[:], sn)
            m2 = ropet.tile([128, 512], bf16, tag="m2", name="m2")
            n2 = ropet.tile([128, 512], bf16, tag="n2", name="n2")
            nc.vector.tensor_mul(m2[:], od[:], sn)
            nc.vector.tensor_mul(n2[:], od[:], c)
            nc.vector.tensor_sub(dst0[0:64, qs], m1[0:64, :], m2[0:64, :])
            nc.vector.tensor_sub(dst1[0:64, qs], m1[64:128, :], m2[64:128, :])
            nc.vector.tensor_add(dst0[64:128, qs], n1[0:64, :], n2[0:64, :])
            nc.vector.tensor_add(dst1[64:128, qs], n1[64:128, :], n2[64:128, :])

        def proj_chunk1(qc, wq_all, wk_all, wv_all, xts):
            """Pass1: QK pair0 + V projection matmuls for one q-chunk.

            xts: this chunk's 16 x tiles [128,512].

            Chunk 0 runs half-major (e-tiles 0-7 fully used before 8-15, so
            the matmuls track the DMA arrival order); later chunks run
            accumulator-major with the rope combines emitted as soon as each
            even/odd accumulator pair closes, spreading the DVE work across
            the chunk instead of bunching it at the boundary.
            """
            qs = slice(qc * 512, (qc + 1) * 512)
            qe = psum.tile([128, 512], f32, tag="t0", name="qe")
            qo = psum.tile([128, 512], f32, tag="t1", name="qo")
            ke = psum.tile([128, 512], f32, tag="t2", name="ke")
            ko = psum.tile([128, 512], f32, tag="t3", name="ko")
            vacc = [psum.tile([128, DL], f32, tag=f"t{4 + i}",
                              name=f"vacc{i}") for i in range(4)]
            for half in range(2):
                ets = range(half * 8, half * 8 + 8)
                for acc, w_all, coff in (
                    (qe, wq_all, 0), (qo, wq_all, 128),
                    (ke, wk_all, 0), (ko, wk_all, 128),
                ):
                    for et in ets:
                        nc.tensor.matmul(
                            acc[:],
                            w_all[:, et * 256 + coff: et * 256 + coff + 128],
                            xts[et][:],
                            start=(et == 0), stop=(et == ET - 1),
                        )
                for sl in range(4):
                    for et in ets:
                        nc.tensor.matmul(
                            vacc[sl][:],
                            xts[et][:, sl * 128:(sl + 1) * 128],
                            wv_all[:, et * DL:(et + 1) * DL],
                            start=(et == 0), stop=(et == ET - 1),
                        )
            rope(QT0[0], QT0[1], qe, qo, qs)
            rope(KT0[0], KT0[1], ke, ko, qs)
            return vacc

        def proj_qk_mms(qc, wq_all, wk_all, QTp, KTp, xts):
            """Pass2 filler: QK pair1 projection, yielding after each matmul.

            Two PSUM banks only (t0/t1), with the rope combines emitted inline
            once each even/odd accumulator pair closes; t2/t3 stay free for
            the attention denominator matmuls.
            """
            qs = slice(qc * 512, (qc + 1) * 512)
            for w_all, dsts in ((wq_all, QTp), (wk_all, KTp)):
                ev = psum.tile([128, 512], f32, tag="t0", name="ev")
                od = psum.tile([128, 512], f32, tag="t1", name="od")
                for acc, coff in ((ev, 0), (od, 128)):
                    for et in range(ET):
                        nc.tensor.matmul(
                            acc[:],
                            w_all[:, et * 256 + coff: et * 256 + coff + 128],
                            xts[et][:],
                            start=(et == 0), stop=(et == ET - 1),
                            skip_group_check=True,
                        )
                        yield
                rope(dsts[0], dsts[1], ev, od, qs)

        def outproj_mms(qcp, wo_h):
            """Pass3 filler: output projection for chunk qcp, yield per matmul."""
            qs = slice(qcp * 512, (qcp + 1) * 512)
            for et in range(ET):
                facc = psum.tile([128, 512], f32, tag=f"t{et % 2}",
                                 name="facc")
                for hh in range(H_CORE):
                    nc.tensor.matmul(
                        facc[:],
                        wo_h[hh][:, et * 128:(et + 1) * 128],
                        OT[hh][:, qs],
                        start=(hh == 0), stop=(hh == H_CORE - 1),
                        skip_group_check=True,
                    )
                    yield
                st = stgp.tile([128, 512], bf16, tag="stg", name="st")
                nc.vector.tensor_copy(st[:], facc[:])
                eng = nc.sync if et % 2 == 0 else nc.gpsimd
                eng.dma_start(outT_d[et * 128:(et + 1) * 128, qs], st[:])

        def attn_gen(QTp, KTp, p, hi, qc, oacc_tag, den_tag, tails):
            """Attention for head (pair p, index hi) on q-chunk qc.

            Yields once per k-tile so the caller can interleave one filler
            matmul per slot.  Softmax denominator: DVE fp16 accumulation of
            the exp tiles, then one all-ones stationary matmul that reduces
            across partitions and broadcasts in the same pass.  The final
            den/recip/normalize tail is appended to `tails` for the caller
            to emit a few slots into the NEXT head (PSUM handoff slack).
            """
            h = 2 * p + hi
            nkt = 4 * qc + 4
            LAG = 2
            qs = slice(qc * 512, (qc + 1) * 512)
            oacc = psum.tile([128, 512], f32, tag=oacc_tag, name="oacc")
            ptacc = accp.tile([128, 512], fp16, tag="ptacc", name="ptacc")
            pts = {}

            def consume(kt):
                j = kt - 4 * qc
                o = 128 * j if j > 0 else 0
                cs = slice(o, 512)
                pt = pts.pop(kt)
                nc.tensor.matmul(
                    oacc[:, cs], V[kt][:, h * 128:(h + 1) * 128], pt[:, cs],
                    start=(kt == 0), stop=(kt == nkt - 1),
                    skip_group_check=True,
                )

            for kt in range(nkt):
                j = kt - 4 * qc
                o = 128 * j if j > 0 else 0
                cs = slice(o, 512)
                sps = psum.tile(
                    [128, 512], f32, tag=("t4" if kt % 2 == 0 else "t5"),
                    name="sps",
                )
                nc.tensor.matmul(
                    sps[:, cs],
                    KTp[hi][:, kt * 128:(kt + 1) * 128],
                    QTp[hi][:, qc * 512 + o:(qc + 1) * 512],
                    start=True, stop=True,
                    skip_group_check=True,
                )
                pt = ptp.tile([128, 512], fp16, tag="pt", name="pt")
                nc.scalar.activation(pt[:, cs], sps[:, cs], Exp, scale=SCALE)
                if j >= 0:
                    mb = slice(o, o + 128)
                    nc.vector.tensor_mul(pt[:, mb], pt[:, mb], tri01[:])
                if kt == 0:
                    nc.vector.tensor_copy(ptacc[:], pt[:])
                else:
                    nc.vector.tensor_add(ptacc[:, cs], ptacc[:, cs], pt[:, cs])
                pts[kt] = pt
                if kt >= LAG:
                    consume(kt - LAG)
                yield
            for kt in range(max(0, nkt - LAG), nkt):
                consume(kt)

            def tail():
                denb = psum.tile([128, 512], f32, tag=den_tag, name="denb")
                nc.tensor.matmul(denb[:], ones16[:], ptacc[:],
                                 start=True, stop=True, skip_group_check=True)
                rec = denp.tile([128, 512], f32, tag="rec", name="rec")
                nc.vector.reciprocal_approx_fast(rec[:], denb[:])
                nc.vector.tensor_mul(OT[h][:, qs], oacc[:], rec[:])

            tails.append(tail)

        # ---- warm-up: un-throttle the PE HAM gate during the initial DMAs --
        nc.vector.memset(zstat[:], 0.0)
        nc.vector.memset(zmov[:], 0.0)
        nc.vector.memset(ones_f[:], 1.0)
        nc.vector.tensor_copy(ones16[:], ones_f[:])
        for w in range(4):
            wps = psum.tile([128, 512], f32, tag="t7", name="warm")
            nc.tensor.matmul(wps[:], zstat[:], zmov[:], start=True, stop=True,
                             skip_group_check=True)

        # ---- pass1: V + QK pair0 over one sweep of x ---------------------
        QT0 = [qkp.tile([DK, S], bf16, tag=f"qt0{i}", name=f"qt0{i}")
               for i in range(2)]
        KT0 = [qkp.tile([DK, S], bf16, tag=f"kt0{i}", name=f"kt0{i}")
               for i in range(2)]
        def load_cossin(qc, eng):
            qs = slice(qc * 512, (qc + 1) * 512)
            eng.dma_start(cos2[0:64, qs], cosT_d[:, qs])
            eng.dma_start(cos2[64:128, qs], cosT_d[:, qs])
            eng.dma_start(sin2[0:64, qs], sinT_d[:, qs])
            eng.dma_start(sin2[64:128, qs], sinT_d[:, qs])

        with tc.tile_pool(name="wvp", bufs=1) as wvp:
            # scalar queue: wq0/wk0 interleaved with wv in consumption order
            # (first matmul needs wq0 chunk0; the V matmuls run ~8us later)
            wq0 = wqp.tile([128, ET * 256], bf16, tag="wq", name="wq")
            wk0 = wkp.tile([128, ET * 256], bf16, tag="wk", name="wk")
            wv_all = wvp.tile([128, ET * DL], bf16, tag="wv", name="wv")
            for kk in range(4):
                ws = slice(kk * 1024, (kk + 1) * 1024)
                nc.scalar.dma_start(wq0[:, ws], wqf_d[:, ws])
                nc.scalar.dma_start(wk0[:, ws], wkf_d[:, ws])
                vs = slice(kk * 2048, (kk + 1) * 2048)
                nc.scalar.dma_start(wv_all[:, vs], wvf_d[:, vs])
            xts0 = load_x(0)
            nc.scalar.dma_start(tri01[:], tri_d[:, :])

            wq1 = wk1 = None
            xts = xts0
            Copy = mybir.ActivationFunctionType.Copy
            for qc in range(QC):
                load_cossin(qc, nc.sync)
                vacc = proj_chunk1(qc, wq0, wk0, wv_all, xts)
                if qc + 1 < QC:
                    xts = load_x(qc + 1)
                for sl in range(4):
                    nc.scalar.activation(V[qc * 4 + sl][:], vacc[sl][:], Copy)
                if qc == 0:
                    wq1, wk1 = load_wqk(1)

        # ---- pass2: pair0 attention with QK pair1 proj interleaved -------
        QT1 = [qkp.tile([DK, S], bf16, tag=f"qt1{i}", name=f"qt1{i}")
               for i in range(2)]
        KT1 = [qkp.tile([DK, S], bf16, tag=f"kt1{i}", name=f"kt1{i}")
               for i in range(2)]
        wo_h = []
        for hh in range(H_CORE):
            wt = wop.tile([128, D], bf16, tag=f"wo{hh}", name=f"wo{hh}")
            nc.scalar.dma_start(wt[:, 0:1024], woT_d[hh * 128:(hh + 1) * 128, 0:1024])
            nc.scalar.dma_start(wt[:, 1024:2048], woT_d[hh * 128:(hh + 1) * 128, 1024:2048])
            wo_h.append(wt)

        tails = []

        def drive(gen, filler):
            n = 0
            for _ in gen:
                next(filler, None)
                n += 1
                if n == 3 and tails:
                    tails.pop(0)()

        def drain(filler):
            n = 0
            for _ in filler:
                n += 1
                if n == 3 and tails:
                    tails.pop(0)()
            while tails:
                tails.pop(0)()

        for qc in range(QC):
            xts = load_x(qc)
            filler = proj_qk_mms(qc, wq1, wk1, QT1, KT1, xts)
            for hi in range(2):
                drive(attn_gen(QT0, KT0, 0, hi, qc,
                               "t7" if hi == 0 else "t6",
                               "t2" if hi == 0 else "t3", tails), filler)
            drain(filler)

        # ---- pass3: pair1 attention with output projection interleaved ---
        for qc in range(QC):
            filler = outproj_mms(qc - 1, wo_h) if qc >= 1 else iter(())
            for hi in range(2):
                drive(attn_gen(QT1, KT1, 1, hi, qc,
                               "t7" if hi == 0 else "t6",
                               "t2" if hi == 0 else "t3", tails), filler)
            drain(filler)
        for _ in outproj_mms(QC - 1, wo_h):
            pass

    return nc


_NC = None


def _get_nc():
    global _NC
    if _NC is None:
        _NC = _build()
        _NC.compile()
    return _NC


def _rope_perm_rows():
    """Row permutation applied to wq/wk for one core's 4 heads.

    Per head-pair p: [hA even dims, hB even dims, hA odd dims, hB odd dims]
    so the device sees even/odd deinterleaved, pair-stacked projections.
    Returns indices into the local (4*DK,) head-row block.
    """
    idx = []
    for p in range(2):
        ha, hb = 2 * p, 2 * p + 1
        idx.extend(ha * DK + np.arange(0, DK, 2))
        idx.extend(hb * DK + np.arange(0, DK, 2))
        idx.extend(ha * DK + np.arange(1, DK, 2))
        idx.extend(hb * DK + np.arange(1, DK, 2))
    return np.asarray(idx)


def _host_tables(positions):
    """cos/sin tables (64, S) float32, matching the fp32 reference math."""
    dim_idx = np.arange(0, DK, 2, dtype=np.float32)
    freqs = np.float32(THETA) ** (dim_idx / np.float32(DK))
    angles = positions.astype(np.float32)[:, None] / freqs[None, :]  # (S, 64)
    return (
        np.ascontiguousarray(np.cos(angles).T.astype(np.float32)),
        np.ascontiguousarray(np.sin(angles).T.astype(np.float32)),
    )


def _flat_qk(wT):
    """(D, DL) -> (128, 2*ET*256): cols [p*4096 + et*256 + c]."""
    a = wT.reshape(ET, 128, DL)  # (et, p, dl)
    out = np.empty((128, 2 * ET * 256), dtype=wT.dtype)
    for p in range(2):
        blk = a[:, :, p * 256:(p + 1) * 256]  # (et, 128, 256)
        out[:, p * ET * 256:(p + 1) * ET * 256] = (
            blk.transpose(1, 0, 2).reshape(128, ET * 256)
        )
    return out


def _flat_v(wT):
    """(D, DL) -> (128, ET*DL): cols [et*512 + dl]."""
    a = wT.reshape(ET, 128, DL)
    return np.ascontiguousarray(a.transpose(1, 0, 2).reshape(128, ET * DL))


def _make_in_maps(inputs):
    x = np.asarray(inputs["x"], dtype=np.float32)
    wq = np.asarray(inputs["wq"], dtype=np.float32)
    wk = np.asarray(inputs["wk"], dtype=np.float32)
    wv = np.asarray(inputs["wv"], dtype=np.float32)
    wo = np.asarray(inputs["wo"], dtype=np.float32)
    token_positions = np.asarray(inputs["token_positions"])

    perm = _rope_perm_rows()
    bfc = ml_dtypes.bfloat16

    r = np.arange(128)
    tri01 = (r[:, None] <= r[None, :]).astype(np.float16)

    in_maps = []
    for c in range(N_CORES):
        b = c // 4
        g = c % 4
        rows = slice(g * DL, (g + 1) * DL)
        cosT, sinT = _host_tables(token_positions[b])
        in_maps.append(
            {
                "xT": np.ascontiguousarray(x[b].T).astype(bfc),
                "wqf": _flat_qk(wq[rows][perm].T).astype(bfc),
                "wkf": _flat_qk(wk[rows][perm].T).astype(bfc),
                "wvf": _flat_v(wv[rows].T).astype(bfc),
                "woT": np.ascontiguousarray(wo[:, rows].T).astype(bfc),
                "cosT": cosT,
                "sinT": sinT,
                "tri01": tri01,
            }
        )
    return in_maps


def kernel(x, wq, wk, wv, wo, token_positions):
    nc = _get_nc()
    in_maps = _make_in_maps(
        {
            "x": x,
            "wq": wq,
            "wk": wk,
            "wv": wv,
            "wo": wo,
            "token_positions": token_positions,
        }
    )
    res = run_bass_kernel_spmd(nc, in_maps, list(range(N_CORES)))

    out = np.zeros((B, S, D), dtype=np.float32)
    for c in range(N_CORES):
        out[c // 4] += res.results[c]["outT"].astype(np.float32).T
    return out


# revision 42
# speedup vs baseline: 1.0749x; 1.0335x over previous
"""Multi-head self-attention with RoPE on 8 Trainium2 NeuronCores.

Problem: B=2, S=2048, D_MODEL=2048, 16 heads x d_k=128, causal, RoPE on Q/K.

Sharding (hardcoded): core c -> batch b=c//4, head group g=c%4 (heads 4g..4g+3).
Data parallel on batch, tensor parallel on heads; q/k/v projections column-
sharded, output projection row-sharded with the partial sums reduced on host.

Device kernel, three passes (bf16/fp16 operands, f32 PSUM accum), tuned so the
PE streams nothing but real matmul columns:
  pass1: V proj (all 4 heads) + Q/K proj pair0 fused over one sweep of x.
  pass2: pair0 attention with Q/K proj pair1 matmuls interleaved 1:1 into the
         attention k-tile slots (the proj matmuls fill the PE while the scalar
         engine's exp keeps pace).
  pass3: pair1 attention with the output projection interleaved the same way.
  Softmax denominator: exp tiles are accumulated on the vector engine (fp16);
  one all-ones [128,128] stationary matmul then reduces across partitions AND
  broadcasts the sums to every partition in a single 512-col pass (213ns on
  the PE vs 3.5us for a gpsimd partition reduce).  Each head's denominator/
  normalize tail is deferred into the next head's slot stream so the PSUM
  bank handoff never stalls the PE.
  Causal mask: 0/1 upper-triangular multiply on the diagonal 128x128 block of
  the exp'd tile (vector engine), replacing the -1e9 mask matmul.
  All PSUM->SBUF copies (V, output staging) run on the vector engine so the
  scalar engine does nothing but exp.
  A short zero matmul warm-up stream un-throttles the PE HAM clock gate while
  the initial weight DMAs land.
"""

import sys

sys.path.insert(0, "/opt/trn_rl_repo")

import math

import ml_dtypes
import numpy as np

import concourse.bass as bass
import concourse.mybir as mybir
from concourse import bass_isa
import concourse.tile as tile
from concourse import bacc
from concourse.bass_utils import run_bass_kernel_spmd

f32 = mybir.dt.float32
bf16 = mybir.dt.bfloat16
fp16 = mybir.dt.float16

B = 2
S = 2048
D = 2048
H = 16
DK = 128
H_CORE = 4  # heads per core
DL = H_CORE * DK  # local feature dim 512
ET = D // 128  # 16 e-tiles (contraction over d_model)
QC = S // 512  # 4 q-chunks
THETA = 10000.0
SCALE = 1.0 / math.sqrt(DK)

N_CORES = 8


def _build():
    nc = bacc.Bacc("TRN2", target_bir_lowering=False, debug=False)

    # x, partition-major: [p, et, s] so one DMA moves 4 e-tiles as a
    # contiguous 4KB-per-partition block (4 DMAs per q-chunk instead of 16
    # -- the DMA completion-semaphore pool serializes when many small DMAs
    # are in flight)
    xT_d = nc.dram_tensor("xT", [128, ET, S], bf16, kind="ExternalInput")
    # per-pair flattened qk weights: cols [p*4096 + et*256 + c], c 0:128 even
    # dims (qe stationary), 128:256 odd dims (qo)
    wqf_d = nc.dram_tensor("wqf", [128, 2 * ET * 256], bf16, kind="ExternalInput")
    wkf_d = nc.dram_tensor("wkf", [128, 2 * ET * 256], bf16, kind="ExternalInput")
    # flattened v weights: cols [et*512 + dl]
    wvf_d = nc.dram_tensor("wvf", [128, ET * DL], bf16, kind="ExternalInput")
    woT_d = nc.dram_tensor("woT", [DL, D], bf16, kind="ExternalInput")
    cosT_d = nc.dram_tensor("cosT", [64, S], f32, kind="ExternalInput")
    sinT_d = nc.dram_tensor("sinT", [64, S], f32, kind="ExternalInput")
    tri_d = nc.dram_tensor("tri01", [128, 128], fp16, kind="ExternalInput")
    outT_d = nc.dram_tensor("outT", [D, S], bf16, kind="ExternalOutput")

    Exp = mybir.ActivationFunctionType.Exp

    with tile.TileContext(nc) as tc:
      with tc.tile_pool(name="const", bufs=1) as const, \
           tc.tile_pool(name="persist", bufs=1) as persist, \
           tc.tile_pool(name="qkp", bufs=1) as qkp, \
           tc.tile_pool(name="wqp", bufs=2) as wqp, \
           tc.tile_pool(name="wkp", bufs=2) as wkp, \
           tc.tile_pool(name="xsp", bufs=8) as xsp, \
           tc.tile_pool(name="ropet", bufs=1) as ropet, \
           tc.tile_pool(name="ptp", bufs=6) as ptp, \
           tc.tile_pool(name="accp", bufs=2) as accp, \
           tc.tile_pool(name="denp", bufs=2) as denp, \
           tc.tile_pool(name="stgp", bufs=8) as stgp, \
           tc.tile_pool(name="wop", bufs=1) as wop, \
           tc.tile_pool(name="psum", bufs=1, space="PSUM") as psum:

        V = [persist.tile([128, DL], fp16, tag=f"v{st}", name=f"v{st}")
             for st in range(ET)]
        OT = [persist.tile([DK, S], bf16, tag=f"ot{h}", name=f"ot{h}")
              for h in range(H_CORE)]

        cos2 = const.tile([128, S], f32, tag="cos2", name="cos2")
        sin2 = const.tile([128, S], f32, tag="sin2", name="sin2")
        tri01 = const.tile([128, 128], fp16, tag="tri01", name="tri01")
        zstat = const.tile([128, 128], bf16, tag="zstat", name="zstat")
        zmov = const.tile([128, 512], bf16, tag="zmov", name="zmov")
        ones_f = const.tile([128, 128], f32, tag="ones_f", name="ones_f")
        ones16 = const.tile([128, 128], fp16, tag="ones16", name="ones16")

        def load_wqk(p, chunks=4, eng=None):
            wq_all = wqp.tile([128, ET * 256], bf16, tag="wq", name="wq")
            wk_all = wkp.tile([128, ET * 256], bf16, tag="wk", name="wk")
            cw = 4096 // chunks
            eng = eng or nc.gpsimd
            for kk in range(chunks):
                cs = slice(kk * cw, (kk + 1) * cw)
                ds = slice(p * 4096 + kk * cw, p * 4096 + (kk + 1) * cw)
                eng.dma_start(wq_all[:, cs], wqf_d[:, ds])
                eng.dma_start(wk_all[:, cs], wkf_d[:, ds])
            return wq_all, wk_all

        def load_x(qc):
            """Per-chunk x: four [128, 4x512] tiles (4 e-tiles each),
            alternating sync/gpsimd queues so arrivals match consumption."""
            qs = slice(qc * 512, (qc + 1) * 512)
            xts = []
            for g in range(4):
                xt = xsp.tile([128, 4 * 512], bf16, tag="xs", name="xs")
                eng = nc.sync if g % 2 == 0 else nc.gpsimd
                eng.dma_start(xt[:], xT_d[:, 4 * g:4 * g + 4, qs])
                xts.append(xt)
            return xts

        def xsl(xts, et, lo=0, hi=512):
            """x slice for e-tile et: cols [lo:hi] within its 512 block."""
            return xts[et // 4][:, (et % 4) * 512 + lo:(et % 4) * 512 + hi]

        def rope(dst0, dst1, ev, od, qs):
            """ev/od: PSUM accumulators (128,512), rows [hA;hB].

            Reads ev fully before od so the PSUM banks free in order for the
            next q-chunk's projection.
            """
            c = cos2[:, qs]
            sn = sin2[:, qs]
            m1 = ropet.tile([128, 512], bf16, tag="m1", name="m1")
            n1 = ropet.tile([128, 512], bf16, tag="n1", name="n1")
            nc.vector.tensor_mul(m1[:], ev[:], c)
            nc.vector.tensor_mul(n1[:], ev[:], sn)
            m2 = ropet.tile([128, 512], bf16, tag="m2", name="m2")
            n2 = ropet.tile([128, 512], bf16, tag="n2", name="n2")
            nc.vector.tensor_mul(m2[:], od[:], sn)
            nc.vector.tensor_mul(n2[:], od[:], c)
            nc.vector.tensor_sub(dst0[0:64, qs], m1[0:64, :], m2[0:64, :])
            nc.vector.tensor_sub(dst1[0:64, qs], m1[64:128, :], m2[64:128, :])
            nc.vector.tensor_add(dst0[64:128, qs], n1[0:64, :], n2[0:64, :])
            nc.vector.tensor_add(dst1[64:128, qs], n1[64:128, :], n2[64:128, :])

        def proj_chunk1(qc, wq_all, wk_all, wv_all, xts):
            """Pass1: QK pair0 + V projection matmuls for one q-chunk.

            xts: this chunk's 16 x tiles [128,512].

            Chunk 0 runs half-major (e-tiles 0-7 fully used before 8-15, so
            the matmuls track the DMA arrival order); later chunks run
            accumulator-major with the rope combines emitted as soon as each
            even/odd accumulator pair closes, spreading the DVE work across
            the chunk instead of bunching it at the boundary.
            """
            qs = slice(qc * 512, (qc + 1) * 512)
            qe = psum.tile([128, 512], f32, tag="t0", name="qe")
            qo = psum.tile([128, 512], f32, tag="t1", name="qo")
            ke = psum.tile([128, 512], f32, tag="t2", name="ke")
            ko = psum.tile([128, 512], f32, tag="t3", name="ko")
            vacc = [psum.tile([128, DL], f32, tag=f"t{4 + i}",
                              name=f"vacc{i}") for i in range(4)]
            for half in range(2):
                ets = range(half * 8, half * 8 + 8)
                for acc, w_all, coff in (
                    (qe, wq_all, 0), (qo, wq_all, 128),
                    (ke, wk_all, 0), (ko, wk_all, 128),
                ):
                    for et in ets:
                        nc.tensor.matmul(
                            acc[:],
                            w_all[:, et * 256 + coff: et * 256 + coff + 128],
                            xsl(xts, et),
                            start=(et == 0), stop=(et == ET - 1),
                        )
                for sl in range(4):
                    for et in ets:
                        nc.tensor.matmul(
                            vacc[sl][:],
                            xsl(xts, et, sl * 128, (sl + 1) * 128),
                            wv_all[:, et * DL:(et + 1) * DL],
                            start=(et == 0), stop=(et == ET - 1),
                        )
            rope(QT0[0], QT0[1], qe, qo, qs)
            rope(KT0[0], KT0[1], ke, ko, qs)
            return vacc

        def proj_qk_mms(qc, wq_all, wk_all, QTp, KTp, xts):
            """Pass2 filler: QK pair1 projection, yielding after each matmul.

            Two PSUM banks only (t0/t1), with the rope combines emitted inline
            once each even/odd accumulator pair closes; t2/t3 stay free for
            the attention denominator matmuls.
            """
            qs = slice(qc * 512, (qc + 1) * 512)
            for w_all, dsts in ((wq_all, QTp), (wk_all, KTp)):
                ev = psum.tile([128, 512], f32, tag="t0", name="ev")
                od = psum.tile([128, 512], f32, tag="t1", name="od")
                for acc, coff in ((ev, 0), (od, 128)):
                    for et in range(ET):
                        nc.tensor.matmul(
                            acc[:],
                            w_all[:, et * 256 + coff: et * 256 + coff + 128],
                            xsl(xts, et),
                            start=(et == 0), stop=(et == ET - 1),
                            skip_group_check=True,
                        )
                        yield
                rope(dsts[0], dsts[1], ev, od, qs)

        def outproj_mms(qcp, wo_h):
            """Pass3 filler: output projection for chunk qcp, yield per matmul."""
            qs = slice(qcp * 512, (qcp + 1) * 512)
            for et in range(ET):
                facc = psum.tile([128, 512], f32, tag=f"t{et % 2}",
                                 name="facc")
                for hh in range(H_CORE):
                    nc.tensor.matmul(
                        facc[:],
                        wo_h[hh][:, et * 128:(et + 1) * 128],
                        OT[hh][:, qs],
                        start=(hh == 0), stop=(hh == H_CORE - 1),
                        skip_group_check=True,
                    )
                    yield
                st = stgp.tile([128, 512], bf16, tag="stg", name="st")
                nc.vector.tensor_copy(st[:], facc[:])
                nc.sync.dma_start(outT_d[et * 128:(et + 1) * 128, qs], st[:])

        def attn_gen(QTp, KTp, p, hi, qc, oacc_tag, den_tag, tails):
            """Attention for head (pair p, index hi) on q-chunk qc.

            Yields once per k-tile so the caller can interleave one filler
            matmul per slot.  Softmax denominator: DVE fp16 accumulation of
            the exp tiles, then one all-ones stationary matmul that reduces
            across partitions and broadcasts in the same pass.  The final
            den/recip/normalize tail is appended to `tails` for the caller
            to emit a few slots into the NEXT head (PSUM handoff slack).
            """
            h = 2 * p + hi
            nkt = 4 * qc + 4
            LAG = 2
            qs = slice(qc * 512, (qc + 1) * 512)
            oacc = psum.tile([128, 512], f32, tag=oacc_tag, name="oacc")
            ptacc = accp.tile([128, 512], fp16, tag="ptacc", name="ptacc")
            pts = {}

            def consume(kt):
                j = kt - 4 * qc
                o = 128 * j if j > 0 else 0
                cs = slice(o, 512)
                pt = pts.pop(kt)
                nc.tensor.matmul(
                    oacc[:, cs], V[kt][:, h * 128:(h + 1) * 128], pt[:, cs],
                    start=(kt == 0), stop=(kt == nkt - 1),
                    skip_group_check=True,
                )

            for kt in range(nkt):
                j = kt - 4 * qc
                o = 128 * j if j > 0 else 0
                cs = slice(o, 512)
                sps = psum.tile(
                    [128, 512], f32, tag=("t4" if kt % 2 == 0 else "t5"),
                    name="sps",
                )
                nc.tensor.matmul(
                    sps[:, cs],
                    KTp[hi][:, kt * 128:(kt + 1) * 128],
                    QTp[hi][:, qc * 512 + o:(qc + 1) * 512],
                    start=True, stop=True,
                    skip_group_check=True,
                )
                pt = ptp.tile([128, 512], fp16, tag="pt", name="pt")
                nc.scalar.activation(pt[:, cs], sps[:, cs], Exp, scale=SCALE)
                if j >= 0:
                    mb = slice(o, o + 128)
                    nc.vector.tensor_mul(pt[:, mb], pt[:, mb], tri01[:])
                if kt == 0:
                    nc.vector.tensor_copy(ptacc[:], pt[:])
                else:
                    nc.vector.tensor_add(ptacc[:, cs], ptacc[:, cs], pt[:, cs])
                pts[kt] = pt
                if kt >= LAG:
                    consume(kt - LAG)
                yield
            for kt in range(max(0, nkt - LAG), nkt):
                consume(kt)

            def tail():
                denb = psum.tile([128, 512], f32, tag=den_tag, name="denb")
                nc.tensor.matmul(denb[:], ones16[:], ptacc[:],
                                 start=True, stop=True, skip_group_check=True)
                rec = denp.tile([128, 512], f32, tag="rec", name="rec")
                nc.vector.reciprocal_approx_fast(rec[:], denb[:])
                nc.vector.tensor_mul(OT[h][:, qs], oacc[:], rec[:])

            tails.append(tail)

        # ---- warm-up: un-throttle the PE HAM gate during the initial DMAs --
        nc.vector.memset(zstat[:], 0.0)
        nc.vector.memset(zmov[:], 0.0)
        nc.vector.memset(ones_f[:], 1.0)
        nc.vector.tensor_copy(ones16[:], ones_f[:])
        for w in range(4):
            wps = psum.tile([128, 512], f32, tag="t7", name="warm")
            nc.tensor.matmul(wps[:], zstat[:], zmov[:], start=True, stop=True,
                             skip_group_check=True)

        # ---- pass1: V + QK pair0 over one sweep of x ---------------------
        QT0 = [qkp.tile([DK, S], bf16, tag=f"qt0{i}", name=f"qt0{i}")
               for i in range(2)]
        KT0 = [qkp.tile([DK, S], bf16, tag=f"kt0{i}", name=f"kt0{i}")
               for i in range(2)]
        def load_cossin(qc, eng):
            qs = slice(qc * 512, (qc + 1) * 512)
            eng.dma_start(cos2[0:64, qs], cosT_d[:, qs])
            eng.dma_start(cos2[64:128, qs], cosT_d[:, qs])
            eng.dma_start(sin2[0:64, qs], sinT_d[:, qs])
            eng.dma_start(sin2[64:128, qs], sinT_d[:, qs])

        with tc.tile_pool(name="wvp", bufs=1) as wvp:
            # scalar queue: wq0/wk0 interleaved with wv in consumption order
            # (first matmul needs wq0 chunk0; the V matmuls run ~8us later)
            wq0 = wqp.tile([128, ET * 256], bf16, tag="wq", name="wq")
            wk0 = wkp.tile([128, ET * 256], bf16, tag="wk", name="wk")
            wv_all = wvp.tile([128, ET * DL], bf16, tag="wv", name="wv")
            for kk in range(4):
                ws = slice(kk * 1024, (kk + 1) * 1024)
                nc.scalar.dma_start(wq0[:, ws], wqf_d[:, ws])
                nc.scalar.dma_start(wk0[:, ws], wkf_d[:, ws])
                vs = slice(kk * 2048, (kk + 1) * 2048)
                nc.scalar.dma_start(wv_all[:, vs], wvf_d[:, vs])
            xts0 = load_x(0)
            nc.scalar.dma_start(tri01[:], tri_d[:, :])

            wq1 = wk1 = None
            xts = xts0
            Copy = mybir.ActivationFunctionType.Copy
            for qc in range(QC):
                load_cossin(qc, nc.scalar)
                vacc = proj_chunk1(qc, wq0, wk0, wv_all, xts)
                if qc + 1 < QC:
                    xts = load_x(qc + 1)
                for sl in range(4):
                    nc.scalar.activation(V[qc * 4 + sl][:], vacc[sl][:], Copy)
                if qc == 0:
                    wq1, wk1 = load_wqk(1)

        # ---- pass2: pair0 attention with QK pair1 proj interleaved -------
        QT1 = [qkp.tile([DK, S], bf16, tag=f"qt1{i}", name=f"qt1{i}")
               for i in range(2)]
        KT1 = [qkp.tile([DK, S], bf16, tag=f"kt1{i}", name=f"kt1{i}")
               for i in range(2)]
        wo_h = []
        for hh in range(H_CORE):
            wt = wop.tile([128, D], bf16, tag=f"wo{hh}", name=f"wo{hh}")
            nc.scalar.dma_start(wt[:, 0:1024], woT_d[hh * 128:(hh + 1) * 128, 0:1024])
            nc.scalar.dma_start(wt[:, 1024:2048], woT_d[hh * 128:(hh + 1) * 128, 1024:2048])
            wo_h.append(wt)

        tails = []

        def drive(gen, filler):
            n = 0
            for _ in gen:
                next(filler, None)
                n += 1
                if n == 3 and tails:
                    tails.pop(0)()

        def drain(filler):
            n = 0
            for _ in filler:
                n += 1
                if n == 3 and tails:
                    tails.pop(0)()
            while tails:
                tails.pop(0)()

        for qc in range(QC):
            xts = load_x(qc)
            filler = proj_qk_mms(qc, wq1, wk1, QT1, KT1, xts)
            for hi in range(2):
                drive(attn_gen(QT0, KT0, 0, hi, qc,
                               "t7" if hi == 0 else "t6",
                               "t2" if hi == 0 else "t3", tails), filler)
            drain(filler)

        # ---- pass3: pair1 attention with output projection interleaved ---
        for qc in range(QC):
            filler = outproj_mms(qc - 1, wo_h) if qc >= 1 else iter(())
            for hi in range(2):
                drive(attn_gen(QT1, KT1, 1, hi, qc,
                               "t7" if hi == 0 else "t6",
                               "t2" if hi == 0 else "t3", tails), filler)
            drain(filler)
        for _ in outproj_mms(QC - 1, wo_h):
            pass

    return nc


_NC = None


def _get_nc():
    global _NC
    if _NC is None:
        _NC = _build()
        _NC.compile()
    return _NC


def _rope_perm_rows():
    """Row permutation applied to wq/wk for one core's 4 heads.

    Per head-pair p: [hA even dims, hB even dims, hA odd dims, hB odd dims]
    so the device sees even/odd deinterleaved, pair-stacked projections.
    Returns indices into the local (4*DK,) head-row block.
    """
    idx = []
    for p in range(2):
        ha, hb = 2 * p, 2 * p + 1
        idx.extend(ha * DK + np.arange(0, DK, 2))
        idx.extend(hb * DK + np.arange(0, DK, 2))
        idx.extend(ha * DK + np.arange(1, DK, 2))
        idx.extend(hb * DK + np.arange(1, DK, 2))
    return np.asarray(idx)


def _host_tables(positions):
    """cos/sin tables (64, S) float32, matching the fp32 reference math."""
    dim_idx = np.arange(0, DK, 2, dtype=np.float32)
    freqs = np.float32(THETA) ** (dim_idx / np.float32(DK))
    angles = positions.astype(np.float32)[:, None] / freqs[None, :]  # (S, 64)
    return (
        np.ascontiguousarray(np.cos(angles).T.astype(np.float32)),
        np.ascontiguousarray(np.sin(angles).T.astype(np.float32)),
    )


def _flat_qk(wT):
    """(D, DL) -> (128, 2*ET*256): cols [p*4096 + et*256 + c]."""
    a = wT.reshape(ET, 128, DL)  # (et, p, dl)
    out = np.empty((128, 2 * ET * 256), dtype=wT.dtype)
    for p in range(2):
        blk = a[:, :, p * 256:(p + 1) * 256]  # (et, 128, 256)
        out[:, p * ET * 256:(p + 1) * ET * 256] = (
            blk.transpose(1, 0, 2).reshape(128, ET * 256)
        )
    return out


def _flat_v(wT):
    """(D, DL) -> (128, ET*DL): cols [et*512 + dl]."""
    a = wT.reshape(ET, 128, DL)
    return np.ascontiguousarray(a.transpose(1, 0, 2).reshape(128, ET * DL))


def _make_in_maps(inputs):
    x = np.asarray(inputs["x"], dtype=np.float32)
    wq = np.asarray(inputs["wq"], dtype=np.float32)
    wk = np.asarray(inputs["wk"], dtype=np.float32)
    wv = np.asarray(inputs["wv"], dtype=np.float32)
    wo = np.asarray(inputs["wo"], dtype=np.float32)
    token_positions = np.asarray(inputs["token_positions"])

    perm = _rope_perm_rows()
    bfc = ml_dtypes.bfloat16

    r = np.arange(128)
    tri01 = (r[:, None] <= r[None, :]).astype(np.float16)

    in_maps = []
    for c in range(N_CORES):
        b = c // 4
        g = c % 4
        rows = slice(g * DL, (g + 1) * DL)
        cosT, sinT = _host_tables(token_positions[b])
        in_maps.append(
            {
                "xT": np.ascontiguousarray(
                    x[b].T.reshape(ET, 128, S).transpose(1, 0, 2)
                ).astype(bfc),
                "wqf": _flat_qk(wq[rows][perm].T).astype(bfc),
                "wkf": _flat_qk(wk[rows][perm].T).astype(bfc),
                "wvf": _flat_v(wv[rows].T).astype(bfc),
                "woT": np.ascontiguousarray(wo[:, rows].T).astype(bfc),
                "cosT": cosT,
                "sinT": sinT,
                "tri01": tri01,
            }
        )
    return in_maps


def kernel(x, wq, wk, wv, wo, token_positions):
    nc = _get_nc()
    in_maps = _make_in_maps(
        {
            "x": x,
            "wq": wq,
            "wk": wk,
            "wv": wv,
            "wo": wo,
            "token_positions": token_positions,
        }
    )
    res = run_bass_kernel_spmd(nc, in_maps, list(range(N_CORES)))

    out = np.zeros((B, S, D), dtype=np.float32)
    for c in range(N_CORES):
        out[c // 4] += res.results[c]["outT"].astype(np.float32).T
    return out
